# revision 14
# baseline (speedup 1.0000x reference)
"""Bahdanau attention kernel for Trainium2, 8-core data-parallel.

Shapes (hardcoded): features [256,225,1280] f32, hidden [256,256] f32,
W1 [1280,256], b1 [256], W2 [256,256], b2 [256], V [256,1], bV [1].
Output: context [256,1280] f32.

Sharding: batch dim split across 8 cores (32 per core); parameters
replicated. No collectives.

Per-core algorithm (batch shard of 32, processed in pairs):
  - load features[b] natural layout [L,D] (2 L-chunks of <=128 partitions)
  - PE-transpose 128x128 blocks -> featT [D,L] (fp32r, 1.5 cyc/row)
  - scoreT[u,l] = tanh(W1.T @ featT + (W2.T @ hiddenT + b1 + b2)) with the
    proj_h+bias term applied as the per-partition bias of the ScalarE tanh
  - logits = V.T @ scoreT  (batch-pair moving dim 450 >= 256 so fp32r
    matmuls run at 1 cycle/row)
  - attn = exp(logits) / sum(exp(logits))  (no max-subtraction needed:
    |logits| <= sum|V| so exp is safe in fp32; bV dropped: softmax-invariant)
  - context = attnT.T @ features_natural  (fp32r, N=512 chunks)
"""

import numpy as np

import concourse.bass as bass
import concourse.bacc as bacc
import concourse.tile as tile
import concourse.mybir as mybir
from concourse import masks
from concourse.bass_utils import run_bass_kernel_spmd

B, L, D, H, U = 256, 225, 1280, 256, 256
NCORES = 8
BS = B // NCORES          # 32 batch items per core
L0, L1 = 128, L - 128     # 128 + 97
DK = D // 128             # 10 d-tiles
F32 = mybir.dt.float32
F32R = mybir.dt.float32r
BF16 = mybir.dt.bfloat16
AF = mybir.ActivationFunctionType


def r(ap):
    """View an fp32 AP as float32r (same bits, fast PE path)."""
    return ap.bitcast(F32R)


def build_kernel():
    nc = bacc.Bacc("TRN2", target_bir_lowering=False, debug=False, num_devices=NCORES)

    feat = nc.dram_tensor("features", [BS, L, D], F32, kind="ExternalInput").ap()
    hid = nc.dram_tensor("hidden", [BS, H], F32, kind="ExternalInput").ap()
    w1 = nc.dram_tensor("W1", [D, U], F32, kind="ExternalInput").ap()
    b1 = nc.dram_tensor("b1", [U], F32, kind="ExternalInput").ap()
    w2 = nc.dram_tensor("W2", [H, U], F32, kind="ExternalInput").ap()
    b2 = nc.dram_tensor("b2", [U], F32, kind="ExternalInput").ap()
    v = nc.dram_tensor("V", [U, 1], F32, kind="ExternalInput").ap()
    nc.dram_tensor("bV", [1], F32, kind="ExternalInput")  # softmax-invariant
    ctx_out = nc.dram_tensor("context", [BS, D], F32, kind="ExternalOutput").ap()

    with tile.TileContext(nc) as tc:
        body(tc, feat, hid, w1, b1, w2, b2, v, ctx_out)
    nc.compile()
    return nc


def body(tc, feat, hid, w1, b1, w2, b2, v, ctx_out):
    nc = tc.nc
    from contextlib import ExitStack

    with ExitStack() as ctx:
        const = ctx.enter_context(tc.tile_pool(name="const", bufs=1))
        fnat_pool = ctx.enter_context(tc.tile_pool(name="fnat", bufs=6))
        featT_pool = ctx.enter_context(tc.tile_pool(name="featT", bufs=3))
        score_pool = ctx.enter_context(tc.tile_pool(name="score", bufs=3))
        small = ctx.enter_context(tc.tile_pool(name="small", bufs=2))
        outst_pool = ctx.enter_context(tc.tile_pool(name="outst", bufs=2))
        # One PSUM pool; bank budget (8 total): trp*2 + scp0 + scp1 + lgp + cxp*2 = 7
        pp = ctx.enter_context(tc.tile_pool(name="pp", bufs=1, space="PSUM"))

        # ---- prefetch first feature loads (before const DMAs so the
        # SWDGE queue serves them first; PE needs fnat before weights) ----
        flat_out = ctx_out.rearrange("b d -> (b d)")
        NPAIR = BS // 2
        st = {}

        def loads(pi):
            fnats = []
            for half in range(2):
                b = 2 * pi + half
                fnatA = fnat_pool.tile([128, D], BF16, tag=f"fnatA{half}",
                                       name=f"fnatA_{pi}_{half}")
                fnatB = fnat_pool.tile([128, D], BF16, tag=f"fnatB{half}",
                                       name=f"fnatB_{pi}_{half}")
                nc.gpsimd.dma_start(fnatA[:, :], feat[b, 0:128, :])
                nc.gpsimd.dma_start(fnatB[0:L1, :], feat[b, 128:L, :])
                fnats.append((fnatA, fnatB))
            st[("fnats", pi)] = fnats

        loads(0)
        loads(1)

        # ---- constants ----
        ident = const.tile([128, 128], F32)
        masks.make_identity(nc, ident[:, :])
        ident_r = const.tile([128, 128], BF16)
        nc.vector.tensor_copy(ident_r[:, :], ident[:, :])

        ones32 = const.tile([1, 32], F32)
        nc.gpsimd.memset(ones32[:, :], 1.0)
        ones32_r = const.tile([1, 32], BF16)
        nc.vector.tensor_copy(ones32_r[:, :], ones32[:, :])

        w2_sb = const.tile([128, 2, U], BF16)   # [h_in_tile, h_tile, u]
        nc.gpsimd.dma_start(w2_sb[:, :, :], w2.rearrange("(k p) u -> p k u", p=128))

        w1_sb = const.tile([128, DK, U], BF16)  # [d_in_tile, d_tile, u]
        nc.gpsimd.dma_start(w1_sb[:, :, :], w1.rearrange("(k p) u -> p k u", p=128))

        v_sb = const.tile([128, 2], BF16)       # [u_in_tile, u_tile]
        nc.gpsimd.dma_start(v_sb[:, :], v.rearrange("(t p) o -> p (t o)", p=128))

        bsum = const.tile([1, U], BF16)         # b1 + b2 (both added pre-tanh)
        b1_sb = const.tile([1, U], F32)
        b2_sb = const.tile([1, U], F32)
        nc.sync.dma_start(b1_sb[:, :], b1[None, :])
        nc.sync.dma_start(b2_sb[:, :], b2[None, :])
        nc.vector.tensor_add(bsum[:, :], b1_sb[:, :], b2_sb[:, :])

        # ---- proj_hT [u, b] = W2.T @ hiddenT + (b1+b2) ----
        hid_nat = const.tile([32, H], F32)
        nc.sync.dma_start(hid_nat[:, :], hid[:, :])

        hidT = const.tile([128, 2, BS], BF16)   # [h_in_tile, h_tile, b]
        if True:
            for hk in range(2):
                hp = pp.tile([128, 256], F32, tag="trp", bufs=2)
                nc.tensor.transpose(
                    hp[:, 0:32], hid_nat[0:32, hk * 128:(hk + 1) * 128],
                    ident[0:32, 0:32])
                nc.vector.tensor_copy(hidT[:, hk, :], hp[:, 0:32])
            projhT = const.tile([128, 2 * BS], F32)  # [u_in_tile, ut*32+b]
            for ut in range(2):
                php = pp.tile([128, 256], F32, tag="trp", bufs=2)
                for hk in range(2):
                    nc.tensor.matmul(
                        php[:, 0:32],
                        lhsT=w2_sb[:, hk, ut * 128:(ut + 1) * 128],
                        rhs=hidT[:, hk, :],
                        start=(hk == 0), stop=False)
                nc.tensor.matmul(
                    php[:, 0:32],
                    lhsT=bsum[0:1, ut * 128:(ut + 1) * 128],
                    rhs=ones32_r[0:1, :].opt(),
                    start=False, stop=True)
                nc.vector.tensor_copy(projhT[:, ut * BS:(ut + 1) * BS], php[:, 0:32])

        # ---- main loop over batch pairs, software-pipelined ----
        # head(p): feature transposes, step-1 matmuls, tanh (loads were
        # issued 2 pairs ahead). tail(p): logits, exp, normalized attn,
        # attn transposes; completed batches accumulate in `ready` and
        # flush as context col-groups of 3 (partitions 0/32/64 of one
        # PSUM bank -> concurrent PE matmuls + one wide copy).
        ready = []

        def head(pi):
            fnats = st[("fnats", pi)]
            featT = featT_pool.tile([128, DK, 2 * L], BF16, tag="featT",
                                    name=f"featT_{pi}")
            for k in range(DK):
                trp = pp.tile([128, 452], BF16, tag="trp", bufs=2,
                              name=f"trp_{pi}_{k}")
                for half in range(2):
                    fnatA, fnatB = fnats[half]
                    o = 226 * half
                    nc.tensor.transpose(
                        trp[:, o:o + 128],
                        fnatA[:, k * 128:(k + 1) * 128],
                        ident_r[:, :])
                    nc.tensor.transpose(
                        trp[:, o + 128:o + 128 + L1 + 1],
                        fnatB[0:L1 + 1, k * 128:(k + 1) * 128],
                        ident_r[0:L1 + 1, 0:L1 + 1])
                nc.vector.tensor_copy(
                    featT[:, k, :],
                    trp.rearrange("p (h x) -> p h x", h=2)[:, :, 0:L])

            score_sb = score_pool.tile([128, 2, 2 * L], BF16, tag="score_sb",
                                       name=f"score_{pi}")
            for ut in range(2):
                scp = pp.tile([128, 512], F32, tag=f"scp{ut}", bufs=1,
                              name=f"scp_{pi}_{ut}")
                for k in range(DK):
                    nc.tensor.matmul(
                        scp[:, 0:2 * L],
                        lhsT=w1_sb[:, k, ut * 128:(ut + 1) * 128],
                        rhs=featT[:, k, :],
                        start=(k == 0), stop=(k == DK - 1))
                for half in range(2):
                    b = 2 * pi + half
                    nc.scalar.activation(
                        score_sb[:, ut, half * L:(half + 1) * L],
                        scp[:, half * L:(half + 1) * L],
                        AF.Tanh,
                        bias=projhT[:, ut * BS + b:ut * BS + b + 1])
            st[("score", pi)] = score_sb

        def flush_ctx():
            group = ready[:3]
            del ready[:3]
            out3 = outst_pool.tile([128, D], F32, tag="out_stage",
                                   name=f"outst_{group[0][0]}")
            rows = 32 * len(group)
            for doff, dw in ((0, 512), (512, 512), (1024, 256)):
                cxp = pp.tile([128, 512], F32, tag="cxp", bufs=2,
                              name=f"cxp_{group[0][0]}_{doff}")
                for q, (b, attnT, c0, fnatA, fnatB) in enumerate(group):
                    nc.tensor.matmul(
                        cxp[32 * q:32 * q + 32, 0:dw],
                        lhsT=attnT[0:128, c0:c0 + 32],
                        rhs=fnatA[:, doff:doff + dw],
                        start=True, stop=False)
                    nc.tensor.matmul(
                        cxp[32 * q:32 * q + 32, 0:dw],
                        lhsT=attnT[0:L1, c0 + 32:c0 + 64],
                        rhs=fnatB[0:L1, doff:doff + dw],
                        start=False, stop=True)
                nc.scalar.copy(out3[0:rows, doff:doff + dw],
                               cxp[0:rows, 0:dw])
            for q, (b, attnT, c0, fnatA, fnatB) in enumerate(group):
                nc.sync.dma_start(ctx_out[b:b + 1, :],
                                  out3[32 * q:32 * q + 1, :])

        def tail(pi):
            score_sb = st.pop(("score", pi))
            fnats = st.pop(("fnats", pi))

            lgp = pp.tile([128, 512], F32, tag="lgp", bufs=2,
                          name=f"lgp_{pi}")
            for ut in range(2):
                nc.tensor.matmul(
                    lgp[0:1, 0:2 * L],
                    lhsT=v_sb[:, ut:ut + 1],
                    rhs=score_sb[:, ut, :],
                    start=(ut == 0), stop=(ut == 1))

            expl = small.tile([1, 2 * L], F32, tag="expl", name=f"expl_{pi}")
            esum = small.tile([1, 2], F32, tag="esum", name=f"esum_{pi}")
            for half in range(2):
                nc.scalar.activation(
                    expl[0:1, half * L:(half + 1) * L],
                    lgp[0:1, half * L:(half + 1) * L],
                    AF.Exp,
                    accum_out=esum[0:1, half:half + 1])
            rsum = small.tile([1, 2], F32, tag="rsum", name=f"rsum_{pi}")
            nc.vector.reciprocal(rsum[:, :], esum[:, :])
            attn = small.tile([1, 2 * L], F32, tag="attn", name=f"attn_{pi}")
            for half in range(2):
                nc.vector.tensor_scalar_mul(
                    attn[0:1, half * L:(half + 1) * L],
                    expl[0:1, half * L:(half + 1) * L],
                    rsum[0:1, half:half + 1])

            atp = pp.tile([128, 512], F32, tag="lgp", bufs=2,
                          name=f"atp_{pi}")
            attnT = small.tile([128, 128], BF16, tag="attnT", bufs=3,
                               name=f"attnT_{pi}")
            nc.gpsimd.memset(attnT[:, :], 0.0)
            for half in range(2):
                nc.tensor.transpose(
                    atp[0:128, 2 * half:2 * half + 1],
                    attn[0:1, half * L:half * L + 128],
                    ident[0:1, 0:1])
                nc.tensor.transpose(
                    atp[0:L1, 2 * half + 1:2 * half + 2],
                    attn[0:1, half * L + 128:half * L + L],
                    ident[0:1, 0:1])
                nc.vector.tensor_copy(attnT[0:128, 64 * half:64 * half + 1],
                                      atp[0:128, 2 * half:2 * half + 1])
                nc.vector.tensor_copy(attnT[0:L1, 64 * half + 32:64 * half + 33],
                                      atp[0:L1, 2 * half + 1:2 * half + 2])
                b = 2 * pi + half
                fnatA, fnatB = fnats[half]
                ready.append((b, attnT, 64 * half, fnatA, fnatB))
            if len(ready) >= 3:
                flush_ctx()

        for pi in range(NPAIR):
            if pi + 2 < NPAIR:
                loads(pi + 2)
            head(pi)
            if pi >= 1:
                tail(pi - 1)
        tail(NPAIR - 1)
        while ready:
            flush_ctx()


def _enable_jax_cache():
    try:
        import jax
        jax.config.update("jax_compilation_cache_dir", "/tmp/jax_neff_cache")
        jax.config.update("jax_persistent_cache_min_entry_size_bytes", 0)
        jax.config.update("jax_persistent_cache_min_compile_time_secs", 0)
    except Exception:
        pass


_enable_jax_cache()

_CACHE = {}


def _get_nc():
    if "nc" not in _CACHE:
        _CACHE["nc"] = build_kernel()
    return _CACHE["nc"]


def _run(inputs, trace=False):
    nc = _get_nc()
    in_maps = []
    for c in range(NCORES):
        sl = slice(c * BS, (c + 1) * BS)
        in_maps.append({
            "features": np.ascontiguousarray(inputs["features"][sl]),
            "hidden": np.ascontiguousarray(inputs["hidden"][sl]),
            "W1": np.ascontiguousarray(inputs["W1"]),
            "b1": np.ascontiguousarray(inputs["b1"]),
            "W2": np.ascontiguousarray(inputs["W2"]),
            "b2": np.ascontiguousarray(inputs["b2"]),
            "V": np.ascontiguousarray(inputs["V"]),
            "bV": np.ascontiguousarray(inputs["bV"]),
        })
    res = run_bass_kernel_spmd(nc, in_maps, core_ids=list(range(NCORES)),
                               trace=trace)
    out = np.concatenate([rr["context"] for rr in res.results], axis=0)
    return out, res


def kernel(**inputs):
    out, _ = _run(inputs, trace=False)
    return out


# revision 19
# speedup vs baseline: 1.2226x; 1.2226x over previous
"""Bahdanau attention kernel for Trainium2, 8-core data-parallel.

Shapes (hardcoded): features [256,225,1280] f32, hidden [256,256] f32,
W1 [1280,256], b1 [256], W2 [256,256], b2 [256], V [256,1], bV [1].
Output: context [256,1280] f32.

Sharding: batch dim split across 8 cores (32 per core); parameters
replicated. No collectives.

Per-core algorithm (batch shard of 32, processed in pairs):
  - load features[b] natural layout [L,D] (2 L-chunks of <=128 partitions)
  - PE-transpose 128x128 blocks -> featT [D,L] (fp32r, 1.5 cyc/row)
  - scoreT[u,l] = tanh(W1.T @ featT + (W2.T @ hiddenT + b1 + b2)) with the
    proj_h+bias term applied as the per-partition bias of the ScalarE tanh
  - logits = V.T @ scoreT  (batch-pair moving dim 450 >= 256 so fp32r
    matmuls run at 1 cycle/row)
  - attn = exp(logits) / sum(exp(logits))  (no max-subtraction needed:
    |logits| <= sum|V| so exp is safe in fp32; bV dropped: softmax-invariant)
  - context = attnT.T @ features_natural  (fp32r, N=512 chunks)
"""

import numpy as np

import concourse.bass as bass
import concourse.bacc as bacc
import concourse.tile as tile
import concourse.mybir as mybir
from concourse import masks
from concourse.bass_utils import run_bass_kernel_spmd

B, L, D, H, U = 256, 225, 1280, 256, 256
NCORES = 8
BS = B // NCORES          # 32 batch items per core
L0, L1 = 128, L - 128     # 128 + 97
DK = D // 128             # 10 d-tiles
F32 = mybir.dt.float32
F32R = mybir.dt.float32r
BF16 = mybir.dt.bfloat16
AF = mybir.ActivationFunctionType


def r(ap):
    """View an fp32 AP as float32r (same bits, fast PE path)."""
    return ap.bitcast(F32R)


def build_kernel():
    nc = bacc.Bacc("TRN2", target_bir_lowering=False, debug=False, num_devices=NCORES)

    feat = nc.dram_tensor("features", [BS, L, D], F32, kind="ExternalInput").ap()
    hid = nc.dram_tensor("hidden", [BS, H], F32, kind="ExternalInput").ap()
    w1 = nc.dram_tensor("W1", [D, U], F32, kind="ExternalInput").ap()
    b1 = nc.dram_tensor("b1", [U], F32, kind="ExternalInput").ap()
    w2 = nc.dram_tensor("W2", [H, U], F32, kind="ExternalInput").ap()
    b2 = nc.dram_tensor("b2", [U], F32, kind="ExternalInput").ap()
    v = nc.dram_tensor("V", [U, 1], F32, kind="ExternalInput").ap()
    nc.dram_tensor("bV", [1], F32, kind="ExternalInput")  # softmax-invariant
    ctx_out = nc.dram_tensor("context", [BS, D], F32, kind="ExternalOutput").ap()

    with tile.TileContext(nc) as tc:
        body(tc, feat, hid, w1, b1, w2, b2, v, ctx_out)
    nc.compile()
    return nc


def body(tc, feat, hid, w1, b1, w2, b2, v, ctx_out):
    nc = tc.nc
    from contextlib import ExitStack

    with ExitStack() as ctx:
        const = ctx.enter_context(tc.tile_pool(name="const", bufs=1))
        fnat_pool = ctx.enter_context(tc.tile_pool(name="fnat", bufs=7))
        featT_pool = ctx.enter_context(tc.tile_pool(name="featT", bufs=3))
        score_pool = ctx.enter_context(tc.tile_pool(name="score", bufs=3))
        small = ctx.enter_context(tc.tile_pool(name="small", bufs=2))
        outst_pool = ctx.enter_context(tc.tile_pool(name="outst", bufs=2))
        # One PSUM pool; bank budget (8 total): trp*2 + scp0 + scp1 + lgp + cxp*2 = 7
        pp = ctx.enter_context(tc.tile_pool(name="pp", bufs=1, space="PSUM"))

        # ---- prefetch first feature loads (before const DMAs so the
        # SWDGE queue serves them first; PE needs fnat before weights) ----
        flat_out = ctx_out.rearrange("b d -> (b d)")
        NPAIR = BS // 2
        st = {}

        feat_flat = feat.rearrange("b l d -> (b l) d")

        def loads(pi):
            fnats = []
            for half in range(2):
                b = 2 * pi + half
                fnat2 = fnat_pool.tile([128, 2, D], BF16, tag=f"fnat{half}",
                                       name=f"fnat_{pi}_{half}")
                if b < BS - 1:
                    # one DMA for both L-chunks: row p, chunk t reads
                    # feat[b, t*128+p, :] (t=1, p>=97 overreads into the
                    # next batch's rows -- harmless garbage, in bounds)
                    nc.gpsimd.dma_start(
                        fnat2[:, :, :],
                        feat_flat[b * L:b * L + 256, :].rearrange(
                            "(t p) d -> p t d", p=128))
                else:
                    nc.gpsimd.dma_start(fnat2[:, 0, :], feat[b, 0:128, :])
                    nc.gpsimd.dma_start(fnat2[0:L1, 1, :], feat[b, 128:L, :])
                fnats.append((fnat2[:, 0, :], fnat2[:, 1, :]))
            st[("fnats", pi)] = fnats

        def loads0():
            # pair 0: split per L-chunk so the first transposes can start
            # as soon as the A-chunks land
            fnats = []
            for half in range(2):
                b = half
                fnat2 = fnat_pool.tile([128, 2, D], BF16, tag=f"fnat{half}",
                                       name=f"fnat_0_{half}")
                nc.gpsimd.dma_start(fnat2[:, 0, :], feat[b, 0:128, :])
                nc.gpsimd.dma_start(fnat2[0:L1, 1, :], feat[b, 128:L, :])
                fnats.append((fnat2[:, 0, :], fnat2[:, 1, :]))
            st[("fnats", 0)] = fnats

        loads0()

        # ---- identity / small consts (needed by first transposes) ----
        ident = const.tile([128, 128], F32)
        masks.make_identity(nc, ident[:, :])
        ident_r = const.tile([128, 128], BF16)
        nc.vector.tensor_copy(ident_r[:, :], ident[:, :])

        ones32 = const.tile([1, 32], F32)
        nc.vector.memset(ones32[:, :], 1.0)
        ones32_r = const.tile([1, 32], BF16)
        nc.vector.tensor_copy(ones32_r[:, :], ones32[:, :])

        # ---- weight loads (right after pair-0 features, before the
        # prefetched pairs, so step-1 is never starved of W1) ----
        w1_sb = const.tile([128, DK, U], BF16)  # [d_in_tile, d_tile, u]
        nc.gpsimd.dma_start(w1_sb[:, :, :], w1.rearrange("(k p) u -> p k u", p=128))

        w2_sb = const.tile([128, 2, U], BF16)   # [h_in_tile, h_tile, u]
        nc.gpsimd.dma_start(w2_sb[:, :, :], w2.rearrange("(k p) u -> p k u", p=128))

        v_sb = const.tile([128, 2], BF16)       # [u_in_tile, u_tile]
        nc.gpsimd.dma_start(v_sb[:, :], v.rearrange("(t p) o -> p (t o)", p=128))

        loads(1)
        loads(2)
        loads(3)

        bsum = const.tile([1, U], BF16)         # b1 + b2 (both added pre-tanh)
        b1_sb = const.tile([1, U], F32)
        b2_sb = const.tile([1, U], F32)
        nc.sync.dma_start(b1_sb[:, :], b1[None, :])
        nc.sync.dma_start(b2_sb[:, :], b2[None, :])
        nc.vector.tensor_add(bsum[:, :], b1_sb[:, :], b2_sb[:, :])

        hid_nat = const.tile([32, H], F32)
        nc.sync.dma_start(hid_nat[:, :], hid[:, :])

        hidT = const.tile([128, 2, BS], BF16)   # [h_in_tile, h_tile, b]
        projhT = const.tile([128, 2 * BS], F32)  # [u_in_tile, ut*32+b]

        def prolog_projh():
            # proj_hT [u, b] = W2.T @ hiddenT + (b1+b2); emitted after
            # head(0) so the PE works on feature transposes while the
            # weight DMAs land.
            for hk in range(2):
                hp = pp.tile([128, 256], F32, tag="trp", bufs=2)
                nc.tensor.transpose(
                    hp[:, 0:32], hid_nat[0:32, hk * 128:(hk + 1) * 128],
                    ident[0:32, 0:32])
                nc.vector.tensor_copy(hidT[:, hk, :], hp[:, 0:32])
            for ut in range(2):
                php = pp.tile([128, 256], F32, tag="trp", bufs=2)
                for hk in range(2):
                    nc.tensor.matmul(
                        php[:, 0:32],
                        lhsT=w2_sb[:, hk, ut * 128:(ut + 1) * 128],
                        rhs=hidT[:, hk, :],
                        start=(hk == 0), stop=False)
                nc.tensor.matmul(
                    php[:, 0:32],
                    lhsT=bsum[0:1, ut * 128:(ut + 1) * 128],
                    rhs=ones32_r[0:1, :].opt(),
                    start=False, stop=True)
                nc.vector.tensor_copy(projhT[:, ut * BS:(ut + 1) * BS], php[:, 0:32])

        # ---- main loop over batch pairs, software-pipelined ----
        # head(p): feature transposes, step-1 matmuls, tanh (loads were
        # issued 2 pairs ahead). tail(p): logits, exp, normalized attn,
        # attn transposes; completed batches accumulate in `ready` and
        # flush as context col-groups of 3 (partitions 0/32/64 of one
        # PSUM bank -> concurrent PE matmuls + one wide copy).
        ready = []

        def head(pi):
            fnats = st[("fnats", pi)]
            featT = featT_pool.tile([128, DK, 2 * L], BF16, tag="featT",
                                    name=f"featT_{pi}")
            for k in range(DK):
                trp = pp.tile([128, 452], BF16, tag="trp", bufs=2,
                              name=f"trp_{pi}_{k}")
                for half in range(2):
                    fnatA, fnatB = fnats[half]
                    o = 226 * half
                    nc.tensor.transpose(
                        trp[:, o:o + 128],
                        fnatA[:, k * 128:(k + 1) * 128],
                        ident_r[:, :])
                    nc.tensor.transpose(
                        trp[:, o + 128:o + 128 + L1 + 1],
                        fnatB[0:L1 + 1, k * 128:(k + 1) * 128],
                        ident_r[0:L1 + 1, 0:L1 + 1])
                nc.vector.tensor_copy(
                    featT[:, k, :],
                    trp.rearrange("p (h x) -> p h x", h=2)[:, :, 0:L])

            scps = []
            for ut in range(2):
                scp = pp.tile([128, 512], F32, tag=f"scp{ut}", bufs=1,
                              name=f"scp_{pi}_{ut}")
                for k in range(DK):
                    nc.tensor.matmul(
                        scp[:, 0:2 * L],
                        lhsT=w1_sb[:, k, ut * 128:(ut + 1) * 128],
                        rhs=featT[:, k, :],
                        start=(k == 0), stop=(k == DK - 1))
                scps.append(scp)
            st[("scps", pi)] = scps

        def tanh_part(pi):
            scps = st.pop(("scps", pi))
            score_sb = score_pool.tile([128, 2, 2 * L], BF16, tag="score_sb",
                                       name=f"score_{pi}")
            for ut in range(2):
                for half in range(2):
                    b = 2 * pi + half
                    nc.scalar.activation(
                        score_sb[:, ut, half * L:(half + 1) * L],
                        scps[ut][:, half * L:(half + 1) * L],
                        AF.Tanh,
                        bias=projhT[:, ut * BS + b:ut * BS + b + 1])
            st[("score", pi)] = score_sb

        def flush_ctx():
            group = ready[:3]
            del ready[:3]
            out3 = outst_pool.tile([128, D], F32, tag="out_stage",
                                   name=f"outst_{group[0][0]}")
            rows = 32 * len(group)
            for doff, dw in ((0, 512), (512, 512), (1024, 256)):
                cxp = pp.tile([128, 512], F32, tag="cxp", bufs=2,
                              name=f"cxp_{group[0][0]}_{doff}")
                for q, (b, attnT, c0, fnatA, fnatB) in enumerate(group):
                    nc.tensor.matmul(
                        cxp[32 * q:32 * q + 32, 0:dw],
                        lhsT=attnT[0:128, c0:c0 + 32],
                        rhs=fnatA[:, doff:doff + dw],
                        start=True, stop=False)
                    nc.tensor.matmul(
                        cxp[32 * q:32 * q + 32, 0:dw],
                        lhsT=attnT[0:L1, c0 + 32:c0 + 64],
                        rhs=fnatB[0:L1, doff:doff + dw],
                        start=False, stop=True)
                nc.scalar.copy(out3[0:rows, doff:doff + dw],
                               cxp[0:rows, 0:dw])
            for q, (b, attnT, c0, fnatA, fnatB) in enumerate(group):
                nc.sync.dma_start(ctx_out[b:b + 1, :],
                                  out3[32 * q:32 * q + 1, :])

        def tail(pi):
            score_sb = st.pop(("score", pi))
            fnats = st.pop(("fnats", pi))

            lgp = pp.tile([128, 512], F32, tag="lgp", bufs=2,
                          name=f"lgp_{pi}")
            for ut in range(2):
                nc.tensor.matmul(
                    lgp[0:1, 0:2 * L],
                    lhsT=v_sb[:, ut:ut + 1],
                    rhs=score_sb[:, ut, :],
                    start=(ut == 0), stop=(ut == 1))

            expl = small.tile([1, 2 * L], F32, tag="expl", name=f"expl_{pi}")
            esum = small.tile([1, 2], F32, tag="esum", name=f"esum_{pi}")
            for half in range(2):
                nc.scalar.activation(
                    expl[0:1, half * L:(half + 1) * L],
                    lgp[0:1, half * L:(half + 1) * L],
                    AF.Exp,
                    accum_out=esum[0:1, half:half + 1])
            rsum = small.tile([1, 2], F32, tag="rsum", name=f"rsum_{pi}")
            nc.vector.reciprocal(rsum[:, :], esum[:, :])
            attn = small.tile([1, 2 * L], F32, tag="attn", name=f"attn_{pi}")
            for half in range(2):
                nc.vector.tensor_scalar_mul(
                    attn[0:1, half * L:(half + 1) * L],
                    expl[0:1, half * L:(half + 1) * L],
                    rsum[0:1, half:half + 1])

            atp = pp.tile([128, 512], F32, tag="lgp", bufs=2,
                          name=f"atp_{pi}")
            attnT = small.tile([128, 128], BF16, tag="attnT", bufs=3,
                               name=f"attnT_{pi}")
            nc.vector.memset(attnT[:, :], 0.0)
            for half in range(2):
                nc.tensor.transpose(
                    atp[0:128, 2 * half:2 * half + 1],
                    attn[0:1, half * L:half * L + 128],
                    ident[0:1, 0:1])
                nc.tensor.transpose(
                    atp[0:L1, 2 * half + 1:2 * half + 2],
                    attn[0:1, half * L + 128:half * L + L],
                    ident[0:1, 0:1])
                nc.vector.tensor_copy(attnT[0:128, 64 * half:64 * half + 1],
                                      atp[0:128, 2 * half:2 * half + 1])
                nc.vector.tensor_copy(attnT[0:L1, 64 * half + 32:64 * half + 33],
                                      atp[0:L1, 2 * half + 1:2 * half + 2])
                b = 2 * pi + half
                fnatA, fnatB = fnats[half]
                ready.append((b, attnT, 64 * half, fnatA, fnatB))
            if len(ready) >= 3:
                flush_ctx()

        for pi in range(NPAIR):
            if pi + 4 < NPAIR:
                loads(pi + 4)
            head(pi)
            if pi == 0:
                prolog_projh()
            tanh_part(pi)
            if pi >= 1:
                tail(pi - 1)
        tail(NPAIR - 1)
        while ready:
            flush_ctx()


def _enable_jax_cache():
    try:
        import jax
        jax.config.update("jax_compilation_cache_dir", "/tmp/jax_neff_cache")
        jax.config.update("jax_persistent_cache_min_entry_size_bytes", 0)
        jax.config.update("jax_persistent_cache_min_compile_time_secs", 0)
    except Exception:
        pass


_enable_jax_cache()

_CACHE = {}


def _get_nc():
    if "nc" not in _CACHE:
        _CACHE["nc"] = build_kernel()
    return _CACHE["nc"]


def _run(inputs, trace=False):
    nc = _get_nc()
    in_maps = []
    for c in range(NCORES):
        sl = slice(c * BS, (c + 1) * BS)
        in_maps.append({
            "features": np.ascontiguousarray(inputs["features"][sl]),
            "hidden": np.ascontiguousarray(inputs["hidden"][sl]),
            "W1": np.ascontiguousarray(inputs["W1"]),
            "b1": np.ascontiguousarray(inputs["b1"]),
            "W2": np.ascontiguousarray(inputs["W2"]),
            "b2": np.ascontiguousarray(inputs["b2"]),
            "V": np.ascontiguousarray(inputs["V"]),
            "bV": np.ascontiguousarray(inputs["bV"]),
        })
    res = run_bass_kernel_spmd(nc, in_maps, core_ids=list(range(NCORES)),
                               trace=trace)
    out = np.concatenate([rr["context"] for rr in res.results], axis=0)
    return out, res


def kernel(**inputs):
    out, _ = _run(inputs, trace=False)
    return out


# revision 20
# speedup vs baseline: 1.2230x; 1.0004x over previous
"""Bahdanau attention kernel for Trainium2, 8-core data-parallel.

Shapes (hardcoded): features [256,225,1280] f32, hidden [256,256] f32,
W1 [1280,256], b1 [256], W2 [256,256], b2 [256], V [256,1], bV [1].
Output: context [256,1280] f32.

Sharding: batch dim split across 8 cores (32 per core); parameters
replicated; no collectives.

Per-core pipeline (batch pairs, software-pipelined head/tail):
  - SWDGE cast-DMA loads features natural [L,D] as bf16 (one 3D-AP DMA
    per batch), 4 pairs ahead of compute
  - PE transposes 128x128 blocks -> featT [D,L] bf16 (both L-chunks of a
    pair share one PSUM bank; single wide DVE copy out)
  - scoreT[u,l] = tanh(W1.T @ featT + bias) on ScalarE, where bias =
    proj_hT[:,b] + b1 + b2 is the per-partition ACT bias (proj_hT
    precomputed once via PE)
  - logits = V.T @ scoreT (pair-wide N=450 bf16 matmuls, fp32 psum)
  - softmax without max-subtraction (|logits| <= sum|V|, safe in fp32);
    bV dropped (softmax-invariant); attn normalized on DVE
  - context = attnT.T @ feat_natural with 3 batches col-grouped into one
    PSUM bank (partitions 0/32/64, zero-padded 32-wide attn columns) so
    the matmuls run concurrently and one wide ScalarE copy drains them
"""

import numpy as np

import concourse.bass as bass
import concourse.bacc as bacc
import concourse.tile as tile
import concourse.mybir as mybir
from concourse import masks
from concourse.bass_utils import run_bass_kernel_spmd

B, L, D, H, U = 256, 225, 1280, 256, 256
NCORES = 8
BS = B // NCORES          # 32 batch items per core
L0, L1 = 128, L - 128     # 128 + 97
DK = D // 128             # 10 d-tiles
F32 = mybir.dt.float32
F32R = mybir.dt.float32r
BF16 = mybir.dt.bfloat16
AF = mybir.ActivationFunctionType


def r(ap):
    """View an fp32 AP as float32r (same bits, fast PE path)."""
    return ap.bitcast(F32R)


def build_kernel():
    nc = bacc.Bacc("TRN2", target_bir_lowering=False, debug=False, num_devices=NCORES)

    feat = nc.dram_tensor("features", [BS, L, D], F32, kind="ExternalInput").ap()
    hid = nc.dram_tensor("hidden", [BS, H], F32, kind="ExternalInput").ap()
    w1 = nc.dram_tensor("W1", [D, U], F32, kind="ExternalInput").ap()
    b1 = nc.dram_tensor("b1", [U], F32, kind="ExternalInput").ap()
    w2 = nc.dram_tensor("W2", [H, U], F32, kind="ExternalInput").ap()
    b2 = nc.dram_tensor("b2", [U], F32, kind="ExternalInput").ap()
    v = nc.dram_tensor("V", [U, 1], F32, kind="ExternalInput").ap()
    nc.dram_tensor("bV", [1], F32, kind="ExternalInput")  # softmax-invariant
    ctx_out = nc.dram_tensor("context", [BS, D], F32, kind="ExternalOutput").ap()

    with tile.TileContext(nc) as tc:
        body(tc, feat, hid, w1, b1, w2, b2, v, ctx_out)
    nc.compile()
    return nc


def body(tc, feat, hid, w1, b1, w2, b2, v, ctx_out):
    nc = tc.nc
    from contextlib import ExitStack

    with ExitStack() as ctx:
        const = ctx.enter_context(tc.tile_pool(name="const", bufs=1))
        fnat_pool = ctx.enter_context(tc.tile_pool(name="fnat", bufs=7))
        featT_pool = ctx.enter_context(tc.tile_pool(name="featT", bufs=3))
        score_pool = ctx.enter_context(tc.tile_pool(name="score", bufs=3))
        small = ctx.enter_context(tc.tile_pool(name="small", bufs=2))
        outst_pool = ctx.enter_context(tc.tile_pool(name="outst", bufs=2))
        # One PSUM pool; bank budget (8 total): trp*2 + scp0 + scp1 + lgp + cxp*2 = 7
        pp = ctx.enter_context(tc.tile_pool(name="pp", bufs=1, space="PSUM"))

        # ---- prefetch first feature loads (before const DMAs so the
        # SWDGE queue serves them first; PE needs fnat before weights) ----
        flat_out = ctx_out.rearrange("b d -> (b d)")
        NPAIR = BS // 2
        st = {}

        feat_flat = feat.rearrange("b l d -> (b l) d")

        def loads(pi):
            fnats = []
            for half in range(2):
                b = 2 * pi + half
                fnat2 = fnat_pool.tile([128, 2, D], BF16, tag=f"fnat{half}",
                                       name=f"fnat_{pi}_{half}")
                if b < BS - 1:
                    # one DMA for both L-chunks: row p, chunk t reads
                    # feat[b, t*128+p, :] (t=1, p>=97 overreads into the
                    # next batch's rows -- harmless garbage, in bounds)
                    nc.gpsimd.dma_start(
                        fnat2[:, :, :],
                        feat_flat[b * L:b * L + 256, :].rearrange(
                            "(t p) d -> p t d", p=128))
                else:
                    nc.gpsimd.dma_start(fnat2[:, 0, :], feat[b, 0:128, :])
                    nc.gpsimd.dma_start(fnat2[0:L1, 1, :], feat[b, 128:L, :])
                fnats.append((fnat2[:, 0, :], fnat2[:, 1, :]))
            st[("fnats", pi)] = fnats

        def loads0():
            # pair 0: split per L-chunk so the first transposes can start
            # as soon as the A-chunks land
            fnats = []
            for half in range(2):
                b = half
                fnat2 = fnat_pool.tile([128, 2, D], BF16, tag=f"fnat{half}",
                                       name=f"fnat_0_{half}")
                nc.gpsimd.dma_start(fnat2[:, 0, :], feat[b, 0:128, :])
                nc.gpsimd.dma_start(fnat2[0:L1, 1, :], feat[b, 128:L, :])
                fnats.append((fnat2[:, 0, :], fnat2[:, 1, :]))
            st[("fnats", 0)] = fnats

        loads0()

        # ---- identity / small consts (needed by first transposes) ----
        ident = const.tile([128, 128], F32)
        masks.make_identity(nc, ident[:, :])
        ident_r = const.tile([128, 128], BF16)
        nc.vector.tensor_copy(ident_r[:, :], ident[:, :])

        ones32 = const.tile([1, 32], F32)
        nc.vector.memset(ones32[:, :], 1.0)
        ones32_r = const.tile([1, 32], BF16)
        nc.vector.tensor_copy(ones32_r[:, :], ones32[:, :])

        # ---- weight loads (right after pair-0 features, before the
        # prefetched pairs, so step-1 is never starved of W1) ----
        w1_sb = const.tile([128, DK, U], BF16)  # [d_in_tile, d_tile, u]
        nc.gpsimd.dma_start(w1_sb[:, :, :], w1.rearrange("(k p) u -> p k u", p=128))

        w2_sb = const.tile([128, 2, U], BF16)   # [h_in_tile, h_tile, u]
        nc.gpsimd.dma_start(w2_sb[:, :, :], w2.rearrange("(k p) u -> p k u", p=128))

        v_sb = const.tile([128, 2], BF16)       # [u_in_tile, u_tile]
        nc.gpsimd.dma_start(v_sb[:, :], v.rearrange("(t p) o -> p (t o)", p=128))

        loads(1)
        loads(2)
        loads(3)

        bsum = const.tile([1, U], BF16)         # b1 + b2 (both added pre-tanh)
        b1_sb = const.tile([1, U], F32)
        b2_sb = const.tile([1, U], F32)
        nc.sync.dma_start(b1_sb[:, :], b1[None, :])
        nc.sync.dma_start(b2_sb[:, :], b2[None, :])
        nc.vector.tensor_add(bsum[:, :], b1_sb[:, :], b2_sb[:, :])

        hid_nat = const.tile([32, H], F32)
        nc.sync.dma_start(hid_nat[:, :], hid[:, :])

        hidT = const.tile([128, 2, BS], BF16)   # [h_in_tile, h_tile, b]
        projhT = const.tile([128, 2 * BS], F32)  # [u_in_tile, ut*32+b]

        def prolog_projh():
            # proj_hT [u, b] = W2.T @ hiddenT + (b1+b2); emitted after
            # head(0) so the PE works on feature transposes while the
            # weight DMAs land.
            for hk in range(2):
                hp = pp.tile([128, 256], F32, tag="trp", bufs=2)
                nc.tensor.transpose(
                    hp[:, 0:32], hid_nat[0:32, hk * 128:(hk + 1) * 128],
                    ident[0:32, 0:32])
                nc.vector.tensor_copy(hidT[:, hk, :], hp[:, 0:32])
            for ut in range(2):
                php = pp.tile([128, 256], F32, tag="trp", bufs=2)
                for hk in range(2):
                    nc.tensor.matmul(
                        php[:, 0:32],
                        lhsT=w2_sb[:, hk, ut * 128:(ut + 1) * 128],
                        rhs=hidT[:, hk, :],
                        start=(hk == 0), stop=False)
                nc.tensor.matmul(
                    php[:, 0:32],
                    lhsT=bsum[0:1, ut * 128:(ut + 1) * 128],
                    rhs=ones32_r[0:1, :].opt(),
                    start=False, stop=True)
                nc.vector.tensor_copy(projhT[:, ut * BS:(ut + 1) * BS], php[:, 0:32])

        # ---- main loop over batch pairs, software-pipelined ----
        # head(p): feature transposes, step-1 matmuls, tanh (loads were
        # issued 2 pairs ahead). tail(p): logits, exp, normalized attn,
        # attn transposes; completed batches accumulate in `ready` and
        # flush as context col-groups of 3 (partitions 0/32/64 of one
        # PSUM bank -> concurrent PE matmuls + one wide copy).
        ready = []

        def head(pi):
            fnats = st[("fnats", pi)]
            featT = featT_pool.tile([128, DK, 2 * L], BF16, tag="featT",
                                    name=f"featT_{pi}")
            for k in range(DK):
                trp = pp.tile([128, 452], BF16, tag="trp", bufs=2,
                              name=f"trp_{pi}_{k}")
                for half in range(2):
                    fnatA, fnatB = fnats[half]
                    o = 226 * half
                    nc.tensor.transpose(
                        trp[:, o:o + 128],
                        fnatA[:, k * 128:(k + 1) * 128],
                        ident_r[:, :])
                    nc.tensor.transpose(
                        trp[:, o + 128:o + 128 + L1 + 1],
                        fnatB[0:L1 + 1, k * 128:(k + 1) * 128],
                        ident_r[0:L1 + 1, 0:L1 + 1])
                nc.vector.tensor_copy(
                    featT[:, k, :],
                    trp.rearrange("p (h x) -> p h x", h=2)[:, :, 0:L])

            scps = []
            for ut in range(2):
                scp = pp.tile([128, 512], F32, tag=f"scp{ut}", bufs=1,
                              name=f"scp_{pi}_{ut}")
                for k in range(DK):
                    nc.tensor.matmul(
                        scp[:, 0:2 * L],
                        lhsT=w1_sb[:, k, ut * 128:(ut + 1) * 128],
                        rhs=featT[:, k, :],
                        start=(k == 0), stop=(k == DK - 1))
                scps.append(scp)
            st[("scps", pi)] = scps

        def tanh_part(pi):
            scps = st.pop(("scps", pi))
            score_sb = score_pool.tile([128, 2, 2 * L], BF16, tag="score_sb",
                                       name=f"score_{pi}")
            for ut in range(2):
                for half in range(2):
                    b = 2 * pi + half
                    nc.scalar.activation(
                        score_sb[:, ut, half * L:(half + 1) * L],
                        scps[ut][:, half * L:(half + 1) * L],
                        AF.Tanh,
                        bias=projhT[:, ut * BS + b:ut * BS + b + 1])
            st[("score", pi)] = score_sb

        def flush_ctx():
            group = ready[:3]
            del ready[:3]
            out3 = outst_pool.tile([128, D], F32, tag="out_stage",
                                   name=f"outst_{group[0][0]}")
            rows = 32 * len(group)
            for doff, dw in ((0, 512), (512, 512), (1024, 256)):
                cxp = pp.tile([128, 512], F32, tag="cxp", bufs=2,
                              name=f"cxp_{group[0][0]}_{doff}")
                for q, (b, attnT, c0, fnatA, fnatB) in enumerate(group):
                    nc.tensor.matmul(
                        cxp[32 * q:32 * q + 32, 0:dw],
                        lhsT=attnT[0:128, c0:c0 + 32],
                        rhs=fnatA[:, doff:doff + dw],
                        start=True, stop=False)
                    nc.tensor.matmul(
                        cxp[32 * q:32 * q + 32, 0:dw],
                        lhsT=attnT[0:L1, c0 + 32:c0 + 64],
                        rhs=fnatB[0:L1, doff:doff + dw],
                        start=False, stop=True)
                nc.scalar.copy(out3[0:rows, doff:doff + dw],
                               cxp[0:rows, 0:dw])
            for q, (b, attnT, c0, fnatA, fnatB) in enumerate(group):
                nc.sync.dma_start(ctx_out[b:b + 1, :],
                                  out3[32 * q:32 * q + 1, :])

        def tail(pi):
            score_sb = st.pop(("score", pi))
            fnats = st.pop(("fnats", pi))

            lgp = pp.tile([128, 512], F32, tag="lgp", bufs=2,
                          name=f"lgp_{pi}")
            for ut in range(2):
                nc.tensor.matmul(
                    lgp[0:1, 0:2 * L],
                    lhsT=v_sb[:, ut:ut + 1],
                    rhs=score_sb[:, ut, :],
                    start=(ut == 0), stop=(ut == 1))

            expl = small.tile([1, 2 * L], F32, tag="expl", name=f"expl_{pi}")
            esum = small.tile([1, 2], F32, tag="esum", name=f"esum_{pi}")
            for half in range(2):
                nc.scalar.activation(
                    expl[0:1, half * L:(half + 1) * L],
                    lgp[0:1, half * L:(half + 1) * L],
                    AF.Exp,
                    accum_out=esum[0:1, half:half + 1])
            rsum = small.tile([1, 2], F32, tag="rsum", name=f"rsum_{pi}")
            nc.vector.reciprocal(rsum[:, :], esum[:, :])
            attn = small.tile([1, 2 * L], F32, tag="attn", name=f"attn_{pi}")
            for half in range(2):
                nc.vector.tensor_scalar_mul(
                    attn[0:1, half * L:(half + 1) * L],
                    expl[0:1, half * L:(half + 1) * L],
                    rsum[0:1, half:half + 1])

            atp = pp.tile([128, 512], F32, tag="lgp", bufs=2,
                          name=f"atp_{pi}")
            attnT = small.tile([128, 128], BF16, tag="attnT", bufs=3,
                               name=f"attnT_{pi}")
            nc.vector.memset(attnT[:, :], 0.0)
            for half in range(2):
                nc.tensor.transpose(
                    atp[0:128, 2 * half:2 * half + 1],
                    attn[0:1, half * L:half * L + 128],
                    ident[0:1, 0:1])
                nc.tensor.transpose(
                    atp[0:L1, 2 * half + 1:2 * half + 2],
                    attn[0:1, half * L + 128:half * L + L],
                    ident[0:1, 0:1])
                nc.vector.tensor_copy(attnT[0:128, 64 * half:64 * half + 1],
                                      atp[0:128, 2 * half:2 * half + 1])
                nc.vector.tensor_copy(attnT[0:L1, 64 * half + 32:64 * half + 33],
                                      atp[0:L1, 2 * half + 1:2 * half + 2])
                b = 2 * pi + half
                fnatA, fnatB = fnats[half]
                ready.append((b, attnT, 64 * half, fnatA, fnatB))
            if len(ready) >= 3:
                flush_ctx()

        for pi in range(NPAIR):
            if pi + 4 < NPAIR:
                loads(pi + 4)
            head(pi)
            if pi == 0:
                prolog_projh()
            tanh_part(pi)
            if pi >= 1:
                tail(pi - 1)
        tail(NPAIR - 1)
        while ready:
            flush_ctx()


def _enable_jax_cache():
    try:
        import jax
        jax.config.update("jax_compilation_cache_dir", "/tmp/jax_neff_cache")
        jax.config.update("jax_persistent_cache_min_entry_size_bytes", 0)
        jax.config.update("jax_persistent_cache_min_compile_time_secs", 0)
    except Exception:
        pass


_enable_jax_cache()

_CACHE = {}


def _get_nc():
    if "nc" not in _CACHE:
        _CACHE["nc"] = build_kernel()
    return _CACHE["nc"]


def _run(inputs, trace=False):
    nc = _get_nc()
    in_maps = []
    for c in range(NCORES):
        sl = slice(c * BS, (c + 1) * BS)
        in_maps.append({
            "features": np.ascontiguousarray(inputs["features"][sl]),
            "hidden": np.ascontiguousarray(inputs["hidden"][sl]),
            "W1": np.ascontiguousarray(inputs["W1"]),
            "b1": np.ascontiguousarray(inputs["b1"]),
            "W2": np.ascontiguousarray(inputs["W2"]),
            "b2": np.ascontiguousarray(inputs["b2"]),
            "V": np.ascontiguousarray(inputs["V"]),
            "bV": np.ascontiguousarray(inputs["bV"]),
        })
    res = run_bass_kernel_spmd(nc, in_maps, core_ids=list(range(NCORES)),
                               trace=trace)
    out = np.concatenate([rr["context"] for rr in res.results], axis=0)
    return out, res


def kernel(**inputs):
    out, _ = _run(inputs, trace=False)
    return out
